# revision 16
# baseline (speedup 1.0000x reference)
"""DecoderRNN (embedding -> 2x GRU(shared weights) -> vocab Linear -> log_softmax)
as a Bass/Tile kernel on 8 Trainium2 NeuronCores.

Sharding:
  - Embedding: the single needed row emb[token] is sliced host-side (the rest
    of the 206MB table is never touched) and replicated across partitions.
  - GRU: hidden dim 1024 sharded 8-way -> 128 units/core; biases packed as an
    extra input column. Layer 1 needs no communication (x and h0 are known);
    h1 is rebuilt with a tiny AllGather + broadcast DMA before layer 2.
  - Output Linear: CONTRACTION-sharded. Each core holds the host-pre-transposed
    [128 own h-dims x 51200 vocab] slice of W_out and computes partial logits
    on the Tensor engine with its own h2 slice as the moving operand -- so the
    big matmul needs NO h2 exchange and no on-device transpose. Partials are
    AllReduce-summed (one 205KB collective at the very end, uncontended).
  - log_softmax: after the AllReduce every core has the full logits, so the
    stats are computed locally (per-partition max/sum pairs combined
    redundantly on all partitions via a broadcast reload -- no cross-partition
    reduce instruction, no stats collective).

The vocab axis is padded 50257 -> 51200; pad weights are 0 and pad biases
-1e30 so exp() underflows to exactly 0. All compute is f32 on device.
"""

import numpy as np

from concourse import bacc, tile, mybir, bass_utils

NHID = 1024
NOUT = 50257
N_CORES = 8
P = 128
HP = NHID // N_CORES        # 128 hidden units per core
VPAD = 51200                # padded vocab
F = NHID + 1                # 1025: GRU weights with bias packed in last column
NCH = 8                     # vocab chunks streamed to PE
VCH = VPAD // NCH           # 6400 vocab per chunk
NJ = VCH // P               # 50 matmuls (psum columns) per chunk
NCOL = VPAD // P            # 400 = columns of the u-ordered logits buffer
N_FREE = 3                  # W chunks allowed to stream before h1 exchange done
WBUFS = 4
NEG_BIG = -1.0e30

F32 = mybir.dt.float32
Alu = mybir.AluOpType
Act = mybir.ActivationFunctionType

_CACHE = {}
LAST_EXEC_NS = None


def _stt_dot(nc, spool, w_ap, x_ap, acc_ap):
    """acc[p] = sum_f w[p, f] * x[p, f] (fused one-pass DVE op)."""
    prod = spool.tile([P, F], F32, tag="prod")
    nc.vector.scalar_tensor_tensor(
        out=prod[:],
        in0=w_ap,
        scalar=1.0,
        in1=x_ap,
        op0=Alu.mult,
        op1=Alu.mult,
        accum_out=acc_ap,
    )


def _build():
    nc = bacc.Bacc(
        "TRN2", target_bir_lowering=False, debug=False, num_devices=N_CORES
    )

    xe_d = nc.dram_tensor("xe", [P, F], F32, kind="ExternalInput")
    he_d = nc.dram_tensor("he", [P, F], F32, kind="ExternalInput")
    hsl_d = nc.dram_tensor("hsl", [P, 1], F32, kind="ExternalInput")
    wih_d = nc.dram_tensor("wih", [P, 3 * F], F32, kind="ExternalInput")
    whh_d = nc.dram_tensor("whh", [P, 3 * F], F32, kind="ExternalInput")
    wvt_d = nc.dram_tensor("wvt", [P, VPAD], F32, kind="ExternalInput")
    bst_d = nc.dram_tensor("bst", [P, NCOL], F32, kind="ExternalInput")
    logp_d = nc.dram_tensor("logp", [P, NCOL], F32, kind="ExternalOutput")
    hout_d = nc.dram_tensor("hout", [P, 1], F32, kind="ExternalOutput")

    groups = [list(range(N_CORES))]

    with tile.TileContext(nc) as tc:
        with (
            tc.tile_pool(name="persist", bufs=1) as pp,
            tc.tile_pool(name="small", bufs=2) as sp,
            tc.tile_pool(name="scratch", bufs=3) as spool,
            tc.tile_pool(name="wstream", bufs=WBUFS) as wpool,
            tc.tile_pool(name="psum", bufs=4, space="PSUM") as psp,
            tc.tile_pool(name="dram", bufs=1, space="DRAM") as dram,
        ):
            # ---- persistent tiles ----
            wih_t = pp.tile([P, 3 * F], F32, tag="wih")
            whh_t = pp.tile([P, 3 * F], F32, tag="whh")
            xe_t = pp.tile([P, F], F32, tag="xe")
            he_t = pp.tile([P, F], F32, tag="he")
            ge_t = pp.tile([P, F], F32, tag="ge")    # layer-2 x=h=h1 bcast
            hsl_t = pp.tile([P, 1], F32, tag="hsl")

            i_wih = nc.sync.dma_start(wih_t[:], wih_d[:, :])
            i_whh = nc.sync.dma_start(whh_t[:], whh_d[:, :])
            i_xe = nc.sync.dma_start(xe_t[:], xe_d[:, :])
            i_he = nc.sync.dma_start(he_t[:], he_d[:, :])
            nc.sync.dma_start(hsl_t[:], hsl_d[:, :])
            gate_insts = [i_wih, i_whh, i_xe, i_he]

            nc.vector.memset(ge_t[:, NHID:], 1.0)

            # x = relu(emb[token])  (leaves the packed 1.0 column alone)
            nc.scalar.activation(xe_t[:, :NHID], xe_t[:, :NHID], Act.Relu)

            def gru_layer(x_t, h_t, hsl_ap, lidx):
                gi, gh = [], []
                for g in range(3):
                    a = sp.tile([P, 1], F32, tag=f"gi{g}")
                    _stt_dot(nc, spool, wih_t[:, g * F:(g + 1) * F], x_t[:], a[:])
                    gi.append(a)
                for g in range(3):
                    a = sp.tile([P, 1], F32, tag=f"gh{g}")
                    _stt_dot(nc, spool, whh_t[:, g * F:(g + 1) * F], h_t[:], a[:])
                    gh.append(a)
                r = sp.tile([P, 1], F32, tag="r")
                z = sp.tile([P, 1], F32, tag="z")
                nc.scalar.activation(r[:], gi[0][:], Act.Sigmoid, bias=gh[0][:])
                nc.scalar.activation(z[:], gi[1][:], Act.Sigmoid, bias=gh[1][:])
                tn = sp.tile([P, 1], F32, tag="tn")
                # tn = r * h_n + i_n
                nc.vector.scalar_tensor_tensor(
                    out=tn[:], in0=gh[2][:], scalar=r[:], in1=gi[2][:],
                    op0=Alu.mult, op1=Alu.add,
                )
                n = sp.tile([P, 1], F32, tag="n")
                nc.scalar.activation(n[:], tn[:], Act.Tanh)
                d = sp.tile([P, 1], F32, tag="d")
                nc.vector.tensor_tensor(d[:], hsl_ap, n[:], Alu.subtract)
                hnew = sp.tile([P, 1], F32, tag=f"hnew{lidx}")
                # hnew = z * (h - n) + n
                nc.vector.scalar_tensor_tensor(
                    out=hnew[:], in0=d[:], scalar=z[:], in1=n[:],
                    op0=Alu.mult, op1=Alu.add,
                )
                return hnew

            h1 = gru_layer(xe_t, he_t, hsl_t[:], 0)

            # ---- h1 exchange (the only mid-pipeline collective) ----
            cc_in = dram.tile([HP], F32, tag="ccin")
            cc_out = dram.tile([NHID], F32, tag="ccout")
            nc.scalar.dma_start(cc_in[:], h1[:, 0])
            nc.gpsimd.collective_compute(
                "AllGather", Alu.bypass, replica_groups=groups,
                ins=[cc_in[:].opt()], outs=[cc_out[:].opt()],
            )
            i_ge = nc.scalar.dma_start(
                ge_t[:, :NHID], cc_out[None, :].to_broadcast([P, NHID])
            )

            h2 = gru_layer(ge_t, ge_t, h1[:], 1)
            nc.scalar.dma_start(hout_d[:, :], h2[:])

            # ---- partial logits on the Tensor engine ----
            part_dram = dram.tile([P, NCOL], F32, tag="part")
            full_dram = dram.tile([P, NCOL], F32, tag="full")
            for t in range(NCH):
                wt = wpool.tile([P, VCH], F32, tag="wt")
                dma = nc.sync.dma_start(
                    wt[:], wvt_d[:, t * VCH:(t + 1) * VCH]
                )
                if t < N_FREE:
                    deps = gate_insts
                else:
                    deps = [i_ge]  # don't contend with the h1 exchange
                for g in deps:
                    tile.add_dep_helper(
                        dma.ins, g.ins, sync=True, reason="dma ordering"
                    )
                ps = psp.tile([P, NJ], F32, tag="ps")
                for j in range(NJ):
                    nc.tensor.matmul(
                        ps[:, j:j + 1],
                        wt[:, j * P:(j + 1) * P],
                        h2[:],
                        start=True,
                        stop=True,
                    )
                pl = spool.tile([P, NJ], F32, tag="pl")
                nc.vector.tensor_copy(pl[:], ps[:])
                nc.scalar.dma_start(
                    part_dram[:, t * NJ:(t + 1) * NJ], pl[:]
                )

            # ---- sum partials across cores ----
            nc.gpsimd.collective_compute(
                "AllReduce", Alu.add, replica_groups=groups,
                ins=[part_dram[:].opt()], outs=[full_dram[:].opt()],
            )

            # ---- bias + log_softmax, fully local ----
            lg = pp.tile([P, NCOL], F32, tag="lg")
            nc.scalar.dma_start(lg[:], full_dram[:, :])
            bst = pp.tile([P, NCOL], F32, tag="bst")
            nc.sync.dma_start(bst[:], bst_d[:, :])
            nc.vector.tensor_tensor(lg[:], lg[:], bst[:], Alu.add)

            rowmax = sp.tile([P, 1], F32, tag="rowmax")
            nc.vector.tensor_reduce(
                rowmax[:], lg[:], mybir.AxisListType.X, Alu.max
            )
            negrm = sp.tile([P, 1], F32, tag="negrm")
            nc.vector.tensor_scalar_mul(negrm[:], rowmax[:], -1.0)
            ex = sp.tile([P, NCOL], F32, tag="ex")
            rowsum = sp.tile([P, 1], F32, tag="rowsum")
            nc.scalar.activation(
                ex[:], lg[:], Act.Exp, bias=negrm[:], accum_out=rowsum[:]
            )
            stat2 = sp.tile([P, 2], F32, tag="stat2")
            nc.vector.tensor_copy(stat2[:, 0:1], rowmax[:])
            nc.vector.tensor_copy(stat2[:, 1:2], rowsum[:])

            st_dram = dram.tile([P * 2], F32, tag="st")
            nc.scalar.dma_start(st_dram[:], stat2[:, :])
            strow = sp.tile([P, P, 2], F32, tag="strow")
            nc.scalar.dma_start(
                strow[:], st_dram[None, :].to_broadcast([P, P * 2])
            )
            m_vals = strow[:, :, 0]
            s_vals = strow[:, :, 1]
            neggm = sp.tile([P, 1], F32, tag="neggm")
            nc.vector.tensor_reduce(
                neggm[:], m_vals, mybir.AxisListType.X, Alu.max
            )
            nc.vector.tensor_scalar_mul(neggm[:], neggm[:], -1.0)
            eall = sp.tile([P, P], F32, tag="eall")
            nc.scalar.activation(eall[:], m_vals, Act.Exp, bias=neggm[:])
            seall = sp.tile([P, P], F32, tag="seall")
            nc.vector.tensor_tensor(seall[:], eall[:], s_vals, Alu.mult)
            gs = sp.tile([P, 1], F32, tag="gs")
            nc.vector.tensor_reduce(gs[:], seall[:], mybir.AxisListType.X, Alu.add)
            lgs = sp.tile([P, 1], F32, tag="lgs")
            nc.scalar.activation(lgs[:], gs[:], Act.Ln)
            negc = sp.tile([P, 1], F32, tag="negc")
            # negc = -gm - log(gs)
            nc.vector.tensor_tensor(negc[:], neggm[:], lgs[:], Alu.subtract)

            lp = sp.tile([P, NCOL], F32, tag="lp")
            nc.scalar.activation(lp[:], lg[:], Act.Identity, bias=negc[:])
            nc.scalar.dma_start(logp_d[:, :], lp[:])

    nc.compile()
    return nc


def kernel(token, hidden, emb, w_ih, w_hh, b_ih, b_hh, W_out, b_out):
    import os

    global LAST_EXEC_NS

    token = np.asarray(token)
    hidden = np.asarray(hidden, dtype=np.float32)
    emb = np.asarray(emb, dtype=np.float32)
    w_ih = np.asarray(w_ih, dtype=np.float32)
    w_hh = np.asarray(w_hh, dtype=np.float32)
    b_ih = np.asarray(b_ih, dtype=np.float32)
    b_hh = np.asarray(b_hh, dtype=np.float32)
    W_out = np.asarray(W_out, dtype=np.float32)
    b_out = np.asarray(b_out, dtype=np.float32)

    tok = int(token.reshape(-1)[0])
    x_row = emb[tok]                       # [1024], pre-relu (relu on device)
    h_row = hidden.reshape(NHID)

    xe = np.empty((P, F), np.float32)
    xe[:, :NHID] = x_row
    xe[:, NHID] = 1.0
    he = np.empty((P, F), np.float32)
    he[:, :NHID] = h_row
    he[:, NHID] = 1.0

    # GRU weight shards: [core][128 units, 3 gates, 1024+1]
    wih4 = w_ih.reshape(3, N_CORES, HP, NHID)
    whh4 = w_hh.reshape(3, N_CORES, HP, NHID)
    bih3 = b_ih.reshape(3, N_CORES, HP)
    bhh3 = b_hh.reshape(3, N_CORES, HP)

    # Pre-transposed output weights: WT[h, v], vocab padded with zeros
    WT = np.zeros((NHID, VPAD), np.float32)
    WT[:, :NOUT] = W_out.T
    b_pad = np.full(VPAD, NEG_BIG, np.float32)
    b_pad[:NOUT] = b_out
    # u-ordered bias table: full_dram[p, cj] holds vocab
    # v = (cj // NJ) * VCH + (cj % NJ) * 128 + p
    p_idx = np.arange(P)[:, None]
    cj_idx = np.arange(NCOL)[None, :]
    vmap = (cj_idx // NJ) * VCH + (cj_idx % NJ) * P + p_idx
    bst = np.ascontiguousarray(b_pad[vmap])

    h_slices = h_row.reshape(N_CORES, HP)

    if "nc" not in _CACHE:
        _CACHE["nc"] = _build()
    nc = _CACHE["nc"]

    in_maps = []
    for c in range(N_CORES):
        wih_c = np.concatenate(
            [wih4[:, c].transpose(1, 0, 2), bih3[:, c].T[:, :, None]], axis=2
        )
        whh_c = np.concatenate(
            [whh4[:, c].transpose(1, 0, 2), bhh3[:, c].T[:, :, None]], axis=2
        )
        in_maps.append(
            {
                "xe": xe,
                "he": he,
                "hsl": np.ascontiguousarray(h_slices[c][:, None]),
                "wih": np.ascontiguousarray(wih_c.reshape(P, 3 * F)),
                "whh": np.ascontiguousarray(whh_c.reshape(P, 3 * F)),
                "wvt": np.ascontiguousarray(WT[c * HP:(c + 1) * HP]),
                "bst": bst,
            }
        )

    trace = os.environ.get("KERNEL_TRACE", "0") == "1"
    res = bass_utils.run_bass_kernel_spmd(
        nc, in_maps, core_ids=list(range(N_CORES)), trace=trace
    )
    LAST_EXEC_NS = res.exec_time_ns

    # every core outputs the full u-ordered logp; un-permute core 0's copy
    full = np.empty(VPAD, np.float32)
    full[vmap] = res.results[0]["logp"]
    logp = full[:NOUT].reshape(1, NOUT)
    h_full = np.concatenate(
        [res.results[c]["hout"][:, 0] for c in range(N_CORES)]
    ).reshape(1, 1, NHID)
    return logp, h_full


# revision 18
# speedup vs baseline: 1.5305x; 1.5305x over previous
"""DecoderRNN (embedding -> 2x GRU(shared weights) -> vocab Linear -> log_softmax)
as a Bass/Tile kernel on 8 Trainium2 NeuronCores.

Sharding / design:
  - Embedding: the single needed row emb[token] is sliced host-side (the rest
    of the 206MB table is never read) and replicated across partitions.
  - GRU: hidden dim 1024 sharded 8-way -> 128 units/core; biases packed as an
    extra input column so each gate is one fused dot. Layer 1 needs no
    communication (x and h0 are host-known); h1 is rebuilt with a tiny
    AllGather + broadcast DMA before layer 2, h2 likewise before the big
    matvec. Collectives are latency-bound when they contend with the bulk
    weight stream, so the W stream is gated off during the exchanges.
  - Output Linear: vocab sharded 8-way (padded 50257->51200 so each core owns
    6400 = 128 partitions x 50 rows; pad weights 0 / pad bias -1e30). Weights
    are cast to bf16 host-side (halves HBM traffic; DVE runs 2-input ops at
    2x in bf16); each row's logit is one fused multiply+reduce
    (scalar_tensor_tensor accum) on the Vector engine with f32 accumulation.
  - log_softmax: per-partition (max, sum-exp) pairs from all 8*128 partitions
    are AllGathered (1KB) and combined redundantly on every partition --
    no cross-partition reduce instruction anywhere.
"""

import numpy as np
import ml_dtypes

from concourse import bacc, tile, mybir, bass_utils

NHID = 1024
NOUT = 50257
N_CORES = 8
P = 128
HP = NHID // N_CORES        # 128 hidden units per core
SLOTS = 50                  # vocab rows per partition per core
VSHARD = P * SLOTS          # 6400 vocab entries per core
VPAD = VSHARD * N_CORES     # 51200 padded vocab
F = NHID + 1                # 1025: weights with bias packed in last column
CHUNK = 5                   # vocab slots per streamed W tile
NCHUNK = SLOTS // CHUNK
N_FREE = 2                  # W chunks allowed before the exchanges complete
WBUFS = NCHUNK              # every chunk gets its own buffer (bf16 W fits)
NEG_BIG = -1.0e30

F32 = mybir.dt.float32
BF16 = mybir.dt.bfloat16
Alu = mybir.AluOpType
Act = mybir.ActivationFunctionType

_CACHE = {}
LAST_EXEC_NS = None


def _build():
    nc = bacc.Bacc(
        "TRN2", target_bir_lowering=False, debug=False, num_devices=N_CORES
    )

    xe_d = nc.dram_tensor("xe", [P, F], F32, kind="ExternalInput")
    he_d = nc.dram_tensor("he", [P, F], F32, kind="ExternalInput")
    hsl_d = nc.dram_tensor("hsl", [P, 1], F32, kind="ExternalInput")
    wih_d = nc.dram_tensor("wih", [P, 3 * F], F32, kind="ExternalInput")
    whh_d = nc.dram_tensor("whh", [P, 3 * F], F32, kind="ExternalInput")
    wout_d = nc.dram_tensor("wout", [P, SLOTS * F], BF16, kind="ExternalInput")
    logp_d = nc.dram_tensor("logp", [P, SLOTS], F32, kind="ExternalOutput")
    hout_d = nc.dram_tensor("hout", [P, 1], F32, kind="ExternalOutput")

    groups = [list(range(N_CORES))]

    with tile.TileContext(nc) as tc:
        with (
            tc.tile_pool(name="persist", bufs=1) as pp,
            tc.tile_pool(name="small", bufs=2) as sp,
            tc.tile_pool(name="scratch", bufs=3) as spool,
            tc.tile_pool(name="wstream", bufs=WBUFS) as wpool,
            tc.tile_pool(name="dram", bufs=1, space="DRAM") as dram,
        ):
            # ---- persistent tiles ----
            wih_t = pp.tile([P, 3 * F], F32, tag="wih")
            whh_t = pp.tile([P, 3 * F], F32, tag="whh")
            xe_t = pp.tile([P, F], F32, tag="xe")
            he_t = pp.tile([P, F], F32, tag="he")
            ge_t = pp.tile([P, F], F32, tag="ge")     # layer-2 x=h=h1 bcast
            oe_t = pp.tile([P, F], F32, tag="oe")     # h2 bcast (f32)
            ob_t = pp.tile([P, F], BF16, tag="ob")    # h2 bcast cast to bf16
            hsl_t = pp.tile([P, 1], F32, tag="hsl")
            logits = pp.tile([P, SLOTS], F32, tag="logits")

            i_wih = nc.sync.dma_start(wih_t[:], wih_d[:, :])
            i_whh = nc.sync.dma_start(whh_t[:], whh_d[:, :])
            i_xe = nc.sync.dma_start(xe_t[:], xe_d[:, :])
            i_he = nc.sync.dma_start(he_t[:], he_d[:, :])
            nc.sync.dma_start(hsl_t[:], hsl_d[:, :])
            gate_insts = [i_wih, i_whh, i_xe, i_he]

            nc.vector.memset(ge_t[:, NHID:], 1.0)
            nc.vector.memset(oe_t[:, NHID:], 1.0)

            # x = relu(emb[token])  (leaves the packed 1.0 column alone)
            nc.scalar.activation(xe_t[:, :NHID], xe_t[:, :NHID], Act.Relu)

            def stt_dot(w_ap, x_ap, acc_ap, dt):
                prod = spool.tile([P, F], dt, tag=f"prod{dt}")
                nc.vector.scalar_tensor_tensor(
                    out=prod[:], in0=w_ap, scalar=1.0, in1=x_ap,
                    op0=Alu.mult, op1=Alu.mult, accum_out=acc_ap,
                )

            def gru_layer(x_t, h_t, hsl_ap, lidx):
                gi, gh = [], []
                for g in range(3):
                    a = sp.tile([P, 1], F32, tag=f"gi{g}")
                    stt_dot(wih_t[:, g * F:(g + 1) * F], x_t[:], a[:], F32)
                    gi.append(a)
                for g in range(3):
                    a = sp.tile([P, 1], F32, tag=f"gh{g}")
                    stt_dot(whh_t[:, g * F:(g + 1) * F], h_t[:], a[:], F32)
                    gh.append(a)
                r = sp.tile([P, 1], F32, tag="r")
                z = sp.tile([P, 1], F32, tag="z")
                nc.scalar.activation(r[:], gi[0][:], Act.Sigmoid, bias=gh[0][:])
                nc.scalar.activation(z[:], gi[1][:], Act.Sigmoid, bias=gh[1][:])
                tn = sp.tile([P, 1], F32, tag="tn")
                # tn = r * h_n + i_n
                nc.vector.scalar_tensor_tensor(
                    out=tn[:], in0=gh[2][:], scalar=r[:], in1=gi[2][:],
                    op0=Alu.mult, op1=Alu.add,
                )
                n = sp.tile([P, 1], F32, tag="n")
                nc.scalar.activation(n[:], tn[:], Act.Tanh)
                d = sp.tile([P, 1], F32, tag="d")
                nc.vector.tensor_tensor(d[:], hsl_ap, n[:], Alu.subtract)
                hnew = sp.tile([P, 1], F32, tag=f"hnew{lidx}")
                # hnew = z * (h - n) + n
                nc.vector.scalar_tensor_tensor(
                    out=hnew[:], in0=d[:], scalar=z[:], in1=n[:],
                    op0=Alu.mult, op1=Alu.add,
                )
                return hnew

            def exchange(hsl_tile, tagn, dest_tile):
                cc_in = dram.tile([HP], F32, tag=f"ccin{tagn}")
                cc_out = dram.tile([NHID], F32, tag=f"ccout{tagn}")
                nc.scalar.dma_start(cc_in[:], hsl_tile[:, 0])
                nc.gpsimd.collective_compute(
                    "AllGather", Alu.bypass, replica_groups=groups,
                    ins=[cc_in[:].opt()], outs=[cc_out[:].opt()],
                )
                return nc.scalar.dma_start(
                    dest_tile[:, :NHID], cc_out[None, :].to_broadcast([P, NHID])
                )

            h1 = gru_layer(xe_t, he_t, hsl_t[:], 0)
            exchange(h1, 0, ge_t)
            h2 = gru_layer(ge_t, ge_t, h1[:], 1)
            nc.scalar.dma_start(hout_d[:, :], h2[:])
            i_oe = exchange(h2, 1, oe_t)
            nc.vector.tensor_copy(ob_t[:], oe_t[:])

            # ---- vocab dots, streaming bf16 W ----
            for t in range(NCHUNK):
                wt = wpool.tile([P, CHUNK * F], BF16, tag="wt")
                dma = nc.sync.dma_start(
                    wt[:], wout_d[:, t * CHUNK * F:(t + 1) * CHUNK * F]
                )
                deps = gate_insts if t < N_FREE else [i_oe]
                for g in deps:
                    tile.add_dep_helper(
                        dma.ins, g.ins, sync=True, reason="dma ordering"
                    )
                for j in range(CHUNK):
                    k = t * CHUNK + j
                    stt_dot(
                        wt[:, j * F:(j + 1) * F], ob_t[:],
                        logits[:, k:k + 1], BF16,
                    )

            # ---- log_softmax via per-partition stats pairs ----
            rowmax = sp.tile([P, 1], F32, tag="rowmax")
            nc.vector.tensor_reduce(
                rowmax[:], logits[:], mybir.AxisListType.X, Alu.max
            )
            negrm = sp.tile([P, 1], F32, tag="negrm")
            nc.vector.tensor_scalar_mul(negrm[:], rowmax[:], -1.0)
            ex = pp.tile([P, SLOTS], F32, tag="ex")
            rowsum = sp.tile([P, 1], F32, tag="rowsum")
            nc.scalar.activation(
                ex[:], logits[:], Act.Exp, bias=negrm[:], accum_out=rowsum[:]
            )
            stat2 = sp.tile([P, 2], F32, tag="stat2")
            nc.vector.tensor_copy(stat2[:, 0:1], rowmax[:])
            nc.vector.tensor_copy(stat2[:, 1:2], rowsum[:])

            npairs = N_CORES * P
            st_in = dram.tile([P * 2], F32, tag="st_in")
            st_all = dram.tile([npairs * 2], F32, tag="st_all")
            nc.scalar.dma_start(st_in[:], stat2[:, :])
            nc.gpsimd.collective_compute(
                "AllGather", Alu.bypass, replica_groups=groups,
                ins=[st_in[:].opt()], outs=[st_all[:].opt()],
            )
            strow = pp.tile([P, npairs, 2], F32, tag="strow")
            nc.scalar.dma_start(
                strow[:], st_all[None, :].to_broadcast([P, npairs * 2])
            )
            m_vals = strow[:, :, 0]
            s_vals = strow[:, :, 1]
            neggm = sp.tile([P, 1], F32, tag="neggm")
            nc.vector.tensor_reduce(neggm[:], m_vals, mybir.AxisListType.X, Alu.max)
            nc.vector.tensor_scalar_mul(neggm[:], neggm[:], -1.0)
            eall = pp.tile([P, npairs], F32, tag="eall")
            nc.scalar.activation(eall[:], m_vals, Act.Exp, bias=neggm[:])
            seall = pp.tile([P, npairs], F32, tag="seall")
            nc.vector.tensor_tensor(seall[:], eall[:], s_vals, Alu.mult)
            gs = sp.tile([P, 1], F32, tag="gs")
            nc.vector.tensor_reduce(gs[:], seall[:], mybir.AxisListType.X, Alu.add)
            lgs = sp.tile([P, 1], F32, tag="lgs")
            nc.scalar.activation(lgs[:], gs[:], Act.Ln)
            negc = sp.tile([P, 1], F32, tag="negc")
            # negc = -gm - log(gs)
            nc.vector.tensor_tensor(negc[:], neggm[:], lgs[:], Alu.subtract)

            lp = pp.tile([P, SLOTS], F32, tag="lp")
            nc.scalar.activation(lp[:], logits[:], Act.Identity, bias=negc[:])
            nc.scalar.dma_start(logp_d[:, :], lp[:])

    nc.compile()
    return nc


def kernel(token, hidden, emb, w_ih, w_hh, b_ih, b_hh, W_out, b_out):
    import os

    global LAST_EXEC_NS

    token = np.asarray(token)
    hidden = np.asarray(hidden, dtype=np.float32)
    emb = np.asarray(emb, dtype=np.float32)
    w_ih = np.asarray(w_ih, dtype=np.float32)
    w_hh = np.asarray(w_hh, dtype=np.float32)
    b_ih = np.asarray(b_ih, dtype=np.float32)
    b_hh = np.asarray(b_hh, dtype=np.float32)
    W_out = np.asarray(W_out, dtype=np.float32)
    b_out = np.asarray(b_out, dtype=np.float32)

    tok = int(token.reshape(-1)[0])
    x_row = emb[tok]                       # [1024], pre-relu (relu on device)
    h_row = hidden.reshape(NHID)

    xe = np.empty((P, F), np.float32)
    xe[:, :NHID] = x_row
    xe[:, NHID] = 1.0
    he = np.empty((P, F), np.float32)
    he[:, :NHID] = h_row
    he[:, NHID] = 1.0

    # GRU weight shards: [core][128 units, 3 gates, 1024+1]
    wih4 = w_ih.reshape(3, N_CORES, HP, NHID)
    whh4 = w_hh.reshape(3, N_CORES, HP, NHID)
    bih3 = b_ih.reshape(3, N_CORES, HP)
    bhh3 = b_hh.reshape(3, N_CORES, HP)

    # Output weights, vocab padded to 51200, bias packed in col 1024, bf16
    Wp = np.zeros((VPAD, F), np.float32)
    Wp[:NOUT, :NHID] = W_out
    Wp[:NOUT, NHID] = b_out
    Wp[NOUT:, NHID] = NEG_BIG
    wout_all = Wp.astype(ml_dtypes.bfloat16).reshape(N_CORES, P, SLOTS * F)

    h_slices = h_row.reshape(N_CORES, HP)

    if "nc" not in _CACHE:
        _CACHE["nc"] = _build()
    nc = _CACHE["nc"]

    in_maps = []
    for c in range(N_CORES):
        wih_c = np.concatenate(
            [wih4[:, c].transpose(1, 0, 2), bih3[:, c].T[:, :, None]], axis=2
        )
        whh_c = np.concatenate(
            [whh4[:, c].transpose(1, 0, 2), bhh3[:, c].T[:, :, None]], axis=2
        )
        in_maps.append(
            {
                "xe": xe,
                "he": he,
                "hsl": np.ascontiguousarray(h_slices[c][:, None]),
                "wih": np.ascontiguousarray(wih_c.reshape(P, 3 * F)),
                "whh": np.ascontiguousarray(whh_c.reshape(P, 3 * F)),
                "wout": np.ascontiguousarray(wout_all[c]),
            }
        )

    trace = os.environ.get("KERNEL_TRACE", "0") == "1"
    res = bass_utils.run_bass_kernel_spmd(
        nc, in_maps, core_ids=list(range(N_CORES)), trace=trace
    )
    LAST_EXEC_NS = res.exec_time_ns

    logp = np.concatenate(
        [res.results[c]["logp"].reshape(-1) for c in range(N_CORES)]
    )[:NOUT].reshape(1, NOUT)
    h_full = np.concatenate(
        [res.results[c]["hout"][:, 0] for c in range(N_CORES)]
    ).reshape(1, 1, NHID)
    return logp, h_full
